# revision 4
# baseline (speedup 1.0000x reference)
"""GNN message-passing (gather + segment-sum) Trainium2 kernel.

Strategy (dst-owner sharding, no collectives), v3:
  - Core c owns output nodes [c*NPC, (c+1)*NPC).
  - x is converted to fp16 on HOST and packed as xb[50000, 128]: row k holds
    nodes 2k (cols 0:64) and 2k+1 (cols 64:128).  Device gathers 128B rows at
    256B HBM stride.  A gather call's HBM base picks (block, parity): block =
    pair-index>>15 (int16 index limit), parity = src&1 selects col 0:64/64:128.
  - Gather descriptor cost is ~8.8ns/idx per SWDGE queue (4 queues max) and
    flat in element size, so padded slots are the only lever: sub-lists are
    UNQUANTIZED (size = max over cores of the real count), packed back-to-back
    within each (group, class) unit; only unit tails pad to x128 chunks.
  - Chunks therefore span bucket boundaries.  Instead of tile_position
    (sub-chunk matmuls crash in accumulation chains: FWL row-offset conflict),
    every matmul covers the full 128 slots of a chunk and bucket selection is
    folded into the one-hot: the host prebuilds ohsrc[p, ohcol] = bucket-local
    dst of slot p if the slot belongs to the column's bucket else -1.  A
    boundary chunk gets one ohcol per overlapping bucket.
  - Device per core, per (group, class): one dma_gather -> stag [128, nck, 64]
    fp16; one DVE is_equal builds ALL the unit's one-hot columns COLUMN-MAJOR
    ([part, dstcol, ohcol], packed 2B last dim -> DVE 2x mode) from ohsrc vs
    iota.  PE matmuls BUCKET-major: psum[128 dst, 64 feat] += oh^T @ stag.
    PSUM accumulation groups must be sequential within a bank (HW-verified).
    ACT copies psum[128, G*64] -> SBUF; per-group DMA to HBM out.
  - Output HBM layout [128, NB*64]: (p, b*64+f) = node b*128+p, feature f.
    Host reshapes/transposes the 8 shards -> [100000, 64] f32.
"""

import sys

for _p in ("/opt/trn_rl_repo", "/root/.axon_site/_ro/trn_rl_repo"):
    if _p not in sys.path:
        sys.path.append(_p)

import numpy as np

from concourse import bass, mybir, tile, bacc
from concourse._compat import exact_div, round_up_to_multiple
from concourse.bass_primitives import MemorySpace
from concourse.bass_utils import run_bass_kernel_spmd
import concourse.ap_utils as ap_utils

P = 128
NPAIR = 50000          # pair rows in xb
PAIR_BLOCK = 32768     # pair-index block boundary (int16 range)


def full_cfg():
    return dict(N=100000, D=64, E=1200000, CORES=8, GROUP=8,
                NSWQ=4, STAG_BUFS=8, OH_BUFS=4, SCRATCH=65536, SRC_SORT=1)


def _dma_gather_small(gp, out_ap, in_ap, idxs_ap, num_idxs, num_idxs_reg,
                      elem_size, elem_step, queue_num=0, single_packet=True):
    """nc.gpsimd.dma_gather minus the elem_size_bytes%256 assert.

    The %256 restriction only applies to the transpose path; the
    non-transpose Q7 desc-gen emits one descriptor of elem_size_bytes per
    index for any size.  The HBM stride (elem_step) must still be x256B.
    """
    gp._assert_queue_num(queue_num)
    assert idxs_ap.dtype == mybir.dt.int16
    assert in_ap.dtype == out_ap.dtype
    elem_size_bytes = elem_size * mybir.dt.size(in_ap.dtype)
    assert elem_size_bytes > 0 and elem_size_bytes % 4 == 0
    assert in_ap.space == MemorySpace.DRAM
    assert idxs_ap.space == MemorySpace.SBUF
    assert out_ap.space == MemorySpace.SBUF
    assert ap_utils.ap_is_contiguous(out_ap.ap[1:])
    assert ap_utils.ap_is_contiguous(idxs_ap.ap[1:])
    assert out_ap.ap[0][1] * out_ap.ap[1][1] == round_up_to_multiple(num_idxs, 128)
    assert in_ap.ap[-1][1] == out_ap.ap[-1][1] == elem_size
    assert in_ap.ap[0][0] == elem_step
    stride_bytes = elem_step * mybir.dt.size(in_ap.dtype)
    stride_bytes_256 = exact_div(stride_bytes, 256)
    assert stride_bytes_256 < 256

    _in_ap = gp.lower_ap_dma(in_ap, for_custom_bir_dma=True)
    _idxs_ap = gp.lower_ap(idxs_ap)
    _out_ap = gp.lower_ap(out_ap)
    return gp.add_instruction(
        mybir.InstDMAGatherAnt(
            name=gp.bass.get_next_instruction_name(),
            ins=[
                *_in_ap,
                _idxs_ap,
                gp.lower_val_access(gp.to_reg(num_idxs_reg)),
            ],
            outs=[_out_ap],
            transpose=False,
            num_idxs=num_idxs,
            elem_size=elem_size,
            stride_bytes_256=stride_bytes_256,
            gen_mode=0,
            single_packet=single_packet,
            queue_num=queue_num,
            sbuf_tokens_per_rank=0,
            sbuf_free_dim_per_rank=0,
            sbuf_free_dim_pad_per_rank=0,
            sbuf_byte_offset=0,
        )
    )


def make_layout(edge_index, cfg):
    """Counting-sort edges into the packed SPMD (group, class, bucket) layout."""
    N, CORES, G = cfg["N"], cfg["CORES"], cfg["GROUP"]
    NPC = N // CORES                      # 12500
    NB = -(-NPC // P)                     # 98 buckets per core
    NCLS = 4
    NG = -(-NB // G)

    src = np.asarray(edge_index[0], dtype=np.int64)
    dst = np.asarray(edge_index[1], dtype=np.int64)
    E = src.shape[0]

    core = dst // NPC
    dstl = dst - core * NPC
    bucket = dstl >> 7
    din = (dstl & 127).astype(np.float32)
    pr = (src & 1)
    half = src >> 1
    blk = (half >= PAIR_BLOCK).astype(np.int64)
    lidx = (half - blk * PAIR_BLOCK).astype(np.int16)
    cls = blk * 2 + pr

    # per-(core, bucket, class) counts -> shared sub-list sizes (no quant)
    cid = (core * NB + bucket) * NCLS + cls
    n = np.bincount(cid, minlength=CORES * NB * NCLS).reshape(CORES, NB, NCLS)
    S = n.max(axis=0).astype(np.int64)            # [NB, NCLS]
    S[:, 0] = np.maximum(S[:, 0], 1)              # every bucket gets >=1 slot

    # layout order: [group][class][bucket in group]; units padded to x128
    sub_start = np.zeros((NB, NCLS), dtype=np.int64)
    units = {}                                    # (gi, c) -> (chunk0, nchunks)
    pos = 0
    for gi in range(NG):
        bks = list(range(gi * G, min((gi + 1) * G, NB)))
        for c in range(NCLS):
            u0 = pos
            for b in bks:
                sub_start[b, c] = pos
                pos += int(S[b, c])
            pos = -(-pos // P) * P
            units[(gi, c)] = (u0 // P, (pos - u0) // P)
    T = pos // P

    # one-hot columns: per (gi, c), per bucket h, one column per chunk its
    # sub-list touches.  sched[gi] = [(b, [(c, tl, ohcol), ...]), ...] with
    # tl local to unit (gi, c) and ohcol local to the unit's oh tile.
    # ohspec[(gi, c)] = [(b, tl, ohcol), ...] duplicates that per-unit view.
    sched = {}
    noh = {}                                      # (gi, c) -> n oh columns
    ohmap = {}                                    # (gi, c) -> list[(b, tl)]
    for gi in range(NG):
        bks = list(range(gi * G, min((gi + 1) * G, NB)))
        glist = []
        cnt = {c: 0 for c in range(NCLS)}
        omap = {c: [] for c in range(NCLS)}
        for b in bks:
            ent = []
            for c in range(NCLS):
                s0, ln = int(sub_start[b, c]), int(S[b, c])
                if ln == 0:
                    continue
                t0, _ = units[(gi, c)]
                tlo = s0 // P - t0
                thi = (s0 + ln - 1) // P - t0
                for tl in range(tlo, thi + 1):
                    ent.append((c, tl, cnt[c]))
                    omap[c].append((b, tl))
                    cnt[c] += 1
            glist.append((b, ent))
        sched[gi] = glist
        for c in range(NCLS):
            noh[(gi, c)] = cnt[c]
            ohmap[(gi, c)] = omap[c]
    nckmax = max(nck for (_, nck) in units.values())
    nohmax = max(noh.values())
    TOH = sum(noh.values())
    ohoff = {}                                    # (gi, c) -> global oh col
    pos_oh = 0
    for gi in range(NG):
        for c in range(NCLS):
            ohoff[(gi, c)] = pos_oh
            pos_oh += noh[(gi, c)]

    # per-edge slot assignment
    key = (core * NB + bucket) * NCLS + cls
    if cfg.get("SRC_SORT"):
        perm = np.lexsort((src, key))
    else:
        perm = np.argsort(key, kind="stable")
    rid = key[perm]
    starts = np.r_[0, np.flatnonzero(np.diff(rid)) + 1]
    counts = np.diff(np.r_[starts, E])
    rank = np.arange(E, dtype=np.int64) - np.repeat(starts, counts)
    slot = sub_start[bucket[perm], cls[perm]] + rank
    core_p = core[perm]

    src_arr = np.zeros((CORES, T * P), dtype=np.int16)   # pads gather row 0
    dst_arr = np.full((CORES, T * P), -1.0, dtype=np.float32)
    src_arr[core_p, slot] = lidx[perm]
    dst_arr[core_p, slot] = din[perm]

    # bucket owner per slot (for ohsrc masking)
    owner = np.full(T * P, -1, dtype=np.int64)
    for b in range(NB):
        for c in range(NCLS):
            if S[b, c]:
                owner[sub_start[b, c]:sub_start[b, c] + S[b, c]] = b

    f16 = mybir.dt.np(mybir.dt.float16)
    idx_np = np.empty((CORES, P, T * 8), dtype=np.int16)
    for c in range(CORES):
        w = src_arr[c].reshape(T * 8, 16).T       # [16, 8T]
        idx_np[c] = np.tile(w, (8, 1))

    # ohsrc[core, p, ohcol]: bucket-local dst of slot (chunk(ohcol), p) if the
    # slot belongs to the column's bucket else -1
    dst3 = dst_arr.reshape(CORES, T, P)           # [core, chunk, p]
    own2 = owner.reshape(T, P)                    # [chunk, p]
    ohsrc = np.full((CORES, P, TOH), -1.0, dtype=np.float32)
    for gi in range(NG):
        for c in range(NCLS):
            t0, _ = units[(gi, c)]
            base = ohoff[(gi, c)]
            for j, (b, tl) in enumerate(ohmap[(gi, c)]):
                tg = t0 + tl
                sel = own2[tg] == b               # [p]
                col = np.where(sel, dst3[:, tg, :], -1.0)
                ohsrc[:, :, base + j] = col
    ohsrc_np = ohsrc.astype(f16)

    meta = dict(NPC=NPC, NB=NB, NG=NG, T=T, sub_start=sub_start, S=S,
                units=units, sched=sched, nckmax=nckmax, nohmax=nohmax,
                noh=noh, ohoff=ohoff, TOH=TOH)
    return S, meta, idx_np, ohsrc_np


def build_nc(S, meta, cfg):
    N, D, CORES, G = cfg["N"], cfg["D"], cfg["CORES"], cfg["GROUP"]
    NB, NG, T = meta["NB"], meta["NG"], meta["T"]
    units, sched = meta["units"], meta["sched"]
    noh, ohoff = meta["noh"], meta["ohoff"]
    NCKMAX, NOHMAX, TOH = meta["nckmax"], meta["nohmax"], meta["TOH"]
    f32 = mybir.dt.float32
    f16 = mybir.dt.float16
    NSWQ = cfg.get("NSWQ", 4)

    _gq = [0]
    nc = bacc.Bacc(
        None,
        target_bir_lowering=False,
        dynamic_dma_scratch_size=cfg.get("SCRATCH", 65536),
        num_swdge_queues=NSWQ,
    )
    xb = nc.dram_tensor("xb", [NPAIR, 2 * D], f16, kind="ExternalInput")
    idx_in = nc.dram_tensor("idx", [P, T * 8], mybir.dt.int16, kind="ExternalInput")
    ohsrc_in = nc.dram_tensor("ohsrc", [P, TOH], f16, kind="ExternalInput")
    iota_in = nc.dram_tensor("iota", [P, P, NOHMAX], f16, kind="ExternalInput")
    out = nc.dram_tensor("out", [P, NB * D], f32, kind="ExternalOutput")

    with tile.TileContext(nc) as tc:
        with (
            tc.tile_pool(name="persist", bufs=1) as persist,
            tc.tile_pool(name="stag", bufs=cfg.get("STAG_BUFS", 8)) as stagp,
            tc.tile_pool(name="oh", bufs=cfg.get("OH_BUFS", 4)) as ohp,
            tc.tile_pool(name="psum", bufs=8, space="PSUM") as psump,
        ):
            idx_t = persist.tile([P, T * 8], mybir.dt.int16)
            ohsrc_t = persist.tile([P, TOH], f16)
            iota_t = persist.tile([P, P, NOHMAX], f16)
            outst = persist.tile([P, NB * D], f32)
            nc.sync.dma_start(idx_t[:], idx_in[:])
            nc.sync.dma_start(ohsrc_t[:], ohsrc_in[:])
            nc.sync.dma_start(iota_t[:, :, :], iota_in[:, :, :])

            import contextlib
            reps = cfg.get("REPS", 0)
            loop_cm = tc.For_i(0, reps, 1) if reps else contextlib.nullcontext()
            with loop_cm:
                for gi in range(NG):
                    bks = list(range(gi * G, min((gi + 1) * G, NB)))
                    GA = len(bks)
                    pt = psump.tile([P, G * D], f32, tag="ps", name=f"ps_{gi}")
                    stags, ohs = {}, {}
                    for c in range(4):
                        t0, nck = units[(gi, c)]
                        if nck == 0:
                            continue
                        stag = stagp.tile([P, nck, D], f16, tag="st")
                        stags[c] = stag
                        blk, par = c >> 1, c & 1
                        r0 = blk * PAIR_BLOCK
                        r1 = NPAIR if blk else PAIR_BLOCK
                        if cfg.get("SKIP_GATHER"):
                            nc.gpsimd.memset(stag[:], 0.0)
                        else:
                            gmax = cfg.get("GMAX", 0) or nck
                            for o in range(0, nck, gmax):
                                w = min(gmax, nck - o)
                                q = _gq[0] % NSWQ
                                _gq[0] += 1
                                _dma_gather_small(
                                    nc.gpsimd,
                                    stag[:, o:o + w, :],
                                    xb[r0:r1, par * D:(par + 1) * D],
                                    idx_t[:, (t0 + o) * 8:(t0 + o + w) * 8],
                                    w * P,
                                    w * P,
                                    D,
                                    2 * D,
                                    queue_num=q,
                                    single_packet=False,
                                )
                        # column-major one-hot [part, dstcol, ohcol]: every
                        # operand has a packed 2-byte last dim -> DVE 2x mode
                        nohc = noh[(gi, c)]
                        oh = ohp.tile([P, P, nohc], f16, tag="oh")
                        ohs[c] = oh
                        o0 = ohoff[(gi, c)]
                        if cfg.get("SKIP_OH"):
                            nc.vector.memset(oh[:], 0.0)
                        else:
                            nc.vector.tensor_tensor(
                                out=oh[:],
                                in0=ohsrc_t[:, None, o0:o0 + nohc].to_broadcast(
                                    [P, P, nohc]),
                                in1=iota_t[:, :, :nohc],
                                op=mybir.AluOpType.is_equal,
                            )
                    if cfg.get("SKIP_MM"):
                        if gi == 0:
                            nc.vector.memset(outst[:], 0.0)
                        nc.sync.dma_start(out[:, bks[0] * D:(bks[-1] + 1) * D],
                                          outst[:, bks[0] * D:(bks[-1] + 1) * D])
                        continue
                    for (b, ent) in sched[gi]:
                        h = b - bks[0]
                        for z, (c, tl, ohcol) in enumerate(ent):
                            nc.tensor.matmul(
                                out=pt[:, h * D:(h + 1) * D],
                                lhsT=ohs[c][:, :, ohcol],
                                rhs=stags[c][:, tl, :],
                                start=(z == 0),
                                stop=(z == len(ent) - 1),
                            )
                    c0 = bks[0] * D
                    c1 = (bks[-1] + 1) * D
                    nc.scalar.copy(out=outst[:, c0:c1], in_=pt[:, :GA * D])
                    nc.sync.dma_start(out[:, c0:c1], outst[:, c0:c1])
    nc.finalize()
    return nc


_CACHE = {}


def _get_nc(S, meta, cfg):
    key = (meta["sub_start"].tobytes(), meta["T"], cfg["N"], cfg["D"],
           cfg["CORES"], cfg["GROUP"])
    if key not in _CACHE:
        _CACHE[key] = build_nc(S, meta, cfg)
    return _CACHE[key]


def make_in_maps(x, idx_np, ohsrc_np, cfg, meta):
    CORES = cfg["CORES"]
    f16 = mybir.dt.np(mybir.dt.float16)
    xb = np.ascontiguousarray(
        np.asarray(x, dtype=np.float32).astype(f16).reshape(NPAIR, 2 * cfg["D"])
    )
    iota = np.ascontiguousarray(
        np.broadcast_to(
            np.arange(P, dtype=np.float32)[None, :, None],
            (P, P, meta["nohmax"]),
        ).astype(f16)
    )
    return [
        {"xb": xb, "idx": idx_np[c], "ohsrc": ohsrc_np[c], "iota": iota}
        for c in range(CORES)
    ]


def assemble(shards, meta, cfg):
    N, D, CORES = cfg["N"], cfg["D"], cfg["CORES"]
    NPC, NB = meta["NPC"], meta["NB"]
    full = np.empty((N, D), dtype=np.float32)
    for c in range(CORES):
        arr = shards[c].reshape(P, NB, D).transpose(1, 0, 2).reshape(NB * P, D)
        full[c * NPC:(c + 1) * NPC] = arr[:NPC]
    return full


def kernel(x, edge_index):
    cfg = full_cfg()
    S, meta, idx_np, ohsrc_np = make_layout(edge_index, cfg)
    nc = _get_nc(S, meta, cfg)
    in_maps = make_in_maps(x, idx_np, ohsrc_np, cfg, meta)
    res = run_bass_kernel_spmd(nc, in_maps, core_ids=list(range(cfg["CORES"])))
    shards = [res.results[c]["out"] for c in range(cfg["CORES"])]
    return assemble(shards, meta, cfg)


# revision 6
# speedup vs baseline: 1.1593x; 1.1593x over previous
"""GNN message-passing (gather + segment-sum) Trainium2 kernel.

Strategy (dst-owner sharding, no collectives), v3:
  - Core c owns output nodes [c*NPC, (c+1)*NPC).
  - x is converted to fp16 on HOST and packed as xb[50000, 128]: row k holds
    nodes 2k (cols 0:64) and 2k+1 (cols 64:128).  Device gathers 128B rows at
    256B HBM stride.  A gather call's HBM base picks (block, parity): block =
    pair-index>>15 (int16 index limit), parity = src&1 selects col 0:64/64:128.
  - Gather descriptor cost is ~8.8ns/idx per SWDGE queue (4 queues max) and
    flat in element size, so padded slots are the only lever: sub-lists are
    UNQUANTIZED (size = max over cores of the real count), packed back-to-back
    within each (group, class) unit; only unit tails pad to x128 chunks.
  - Chunks therefore span bucket boundaries.  Instead of tile_position
    (sub-chunk matmuls crash in accumulation chains: FWL row-offset conflict),
    every matmul covers the full 128 slots of a chunk and bucket selection is
    folded into the one-hot: the host prebuilds ohsrc[p, ohcol] = bucket-local
    dst of slot p if the slot belongs to the column's bucket else -1.  A
    boundary chunk gets one ohcol per overlapping bucket.
  - Device per core, per (group, class): one dma_gather -> stag [128, nck, 64]
    fp16; one DVE is_equal builds ALL the unit's one-hot columns COLUMN-MAJOR
    ([part, dstcol, ohcol], packed 2B last dim -> DVE 2x mode) from ohsrc vs
    iota.  PE matmuls BUCKET-major: psum[128 dst, 64 feat] += oh^T @ stag.
    PSUM accumulation groups must be sequential within a bank (HW-verified).
    ACT copies psum[128, G*64] -> SBUF; per-group DMA to HBM out.
  - Output HBM layout [128, NB*64]: (p, b*64+f) = node b*128+p, feature f.
    Host reshapes/transposes the 8 shards -> [100000, 64] f32.
"""

import sys

for _p in ("/opt/trn_rl_repo", "/root/.axon_site/_ro/trn_rl_repo"):
    if _p not in sys.path:
        sys.path.append(_p)

import numpy as np

from concourse import bass, mybir, tile, bacc
from concourse._compat import exact_div, round_up_to_multiple
from concourse.bass_primitives import MemorySpace
from concourse.bass_utils import run_bass_kernel_spmd
import concourse.ap_utils as ap_utils

P = 128
NPAIR = 50000          # pair rows in xb
PAIR_BLOCK = 32768     # pair-index block boundary (int16 range)


def full_cfg():
    return dict(N=100000, D=64, E=1200000, CORES=8, GROUP=8,
                NSWQ=4, STAG_BUFS=8, OH_BUFS=4, SCRATCH=65536, SRC_SORT=1)


def _dma_gather_small(gp, out_ap, in_ap, idxs_ap, num_idxs, num_idxs_reg,
                      elem_size, elem_step, queue_num=0, single_packet=True):
    """nc.gpsimd.dma_gather minus the elem_size_bytes%256 assert.

    The %256 restriction only applies to the transpose path; the
    non-transpose Q7 desc-gen emits one descriptor of elem_size_bytes per
    index for any size.  The HBM stride (elem_step) must still be x256B.
    """
    gp._assert_queue_num(queue_num)
    assert idxs_ap.dtype == mybir.dt.int16
    assert in_ap.dtype == out_ap.dtype
    elem_size_bytes = elem_size * mybir.dt.size(in_ap.dtype)
    assert elem_size_bytes > 0 and elem_size_bytes % 4 == 0
    assert in_ap.space == MemorySpace.DRAM
    assert idxs_ap.space == MemorySpace.SBUF
    assert out_ap.space == MemorySpace.SBUF
    assert ap_utils.ap_is_contiguous(out_ap.ap[1:])
    assert ap_utils.ap_is_contiguous(idxs_ap.ap[1:])
    assert out_ap.ap[0][1] * out_ap.ap[1][1] == round_up_to_multiple(num_idxs, 128)
    assert in_ap.ap[-1][1] == out_ap.ap[-1][1] == elem_size
    assert in_ap.ap[0][0] == elem_step
    stride_bytes = elem_step * mybir.dt.size(in_ap.dtype)
    stride_bytes_256 = exact_div(stride_bytes, 256)
    assert stride_bytes_256 < 256

    _in_ap = gp.lower_ap_dma(in_ap, for_custom_bir_dma=True)
    _idxs_ap = gp.lower_ap(idxs_ap)
    _out_ap = gp.lower_ap(out_ap)
    return gp.add_instruction(
        mybir.InstDMAGatherAnt(
            name=gp.bass.get_next_instruction_name(),
            ins=[
                *_in_ap,
                _idxs_ap,
                gp.lower_val_access(gp.to_reg(num_idxs_reg)),
            ],
            outs=[_out_ap],
            transpose=False,
            num_idxs=num_idxs,
            elem_size=elem_size,
            stride_bytes_256=stride_bytes_256,
            gen_mode=0,
            single_packet=single_packet,
            queue_num=queue_num,
            sbuf_tokens_per_rank=0,
            sbuf_free_dim_per_rank=0,
            sbuf_free_dim_pad_per_rank=0,
            sbuf_byte_offset=0,
        )
    )


def make_layout(edge_index, cfg):
    """Counting-sort edges into the packed SPMD (group, class, bucket) layout."""
    N, CORES, G = cfg["N"], cfg["CORES"], cfg["GROUP"]
    NPC = N // CORES                      # 12500
    NB = -(-NPC // P)                     # 98 buckets per core
    NCLS = 4
    NG = -(-NB // G)

    src = np.asarray(edge_index[0], dtype=np.int64)
    dst = np.asarray(edge_index[1], dtype=np.int64)
    E = src.shape[0]

    core = dst // NPC
    dstl = dst - core * NPC
    bucket = dstl >> 7
    din = (dstl & 127).astype(np.float32)
    pr = (src & 1)
    half = src >> 1
    blk = (half >= PAIR_BLOCK).astype(np.int64)
    lidx = (half - blk * PAIR_BLOCK).astype(np.int16)
    cls = blk * 2 + pr

    # per-(core, bucket, class) counts -> shared sub-list sizes (no quant)
    cid = (core * NB + bucket) * NCLS + cls
    n = np.bincount(cid, minlength=CORES * NB * NCLS).reshape(CORES, NB, NCLS)
    S = n.max(axis=0).astype(np.int64)            # [NB, NCLS]
    S[:, 0] = np.maximum(S[:, 0], 1)              # every bucket gets >=1 slot

    # layout order: [group][class][bucket in group]; units padded to x128
    sub_start = np.zeros((NB, NCLS), dtype=np.int64)
    units = {}                                    # (gi, c) -> (chunk0, nchunks)
    pos = 0
    for gi in range(NG):
        bks = list(range(gi * G, min((gi + 1) * G, NB)))
        for c in range(NCLS):
            u0 = pos
            for b in bks:
                sub_start[b, c] = pos
                pos += int(S[b, c])
            pos = -(-pos // P) * P
            units[(gi, c)] = (u0 // P, (pos - u0) // P)
    T = pos // P

    # one-hot columns: per (gi, c), per bucket h, one column per chunk its
    # sub-list touches.  sched[gi] = [(b, [(c, tl, ohcol), ...]), ...] with
    # tl local to unit (gi, c) and ohcol local to the unit's oh tile.
    # ohspec[(gi, c)] = [(b, tl, ohcol), ...] duplicates that per-unit view.
    sched = {}
    noh = {}                                      # (gi, c) -> n oh columns
    ohmap = {}                                    # (gi, c) -> list[(b, tl)]
    for gi in range(NG):
        bks = list(range(gi * G, min((gi + 1) * G, NB)))
        glist = []
        cnt = {c: 0 for c in range(NCLS)}
        omap = {c: [] for c in range(NCLS)}
        for b in bks:
            ent = []
            for c in range(NCLS):
                s0, ln = int(sub_start[b, c]), int(S[b, c])
                if ln == 0:
                    continue
                t0, _ = units[(gi, c)]
                tlo = s0 // P - t0
                thi = (s0 + ln - 1) // P - t0
                for tl in range(tlo, thi + 1):
                    ent.append((c, tl, cnt[c]))
                    omap[c].append((b, tl))
                    cnt[c] += 1
            glist.append((b, ent))
        sched[gi] = glist
        for c in range(NCLS):
            noh[(gi, c)] = cnt[c]
            ohmap[(gi, c)] = omap[c]
    nckmax = max(nck for (_, nck) in units.values())
    nohmax = max(noh.values())
    TOH = sum(noh.values())
    ohoff = {}                                    # (gi, c) -> global oh col
    pos_oh = 0
    for gi in range(NG):
        for c in range(NCLS):
            ohoff[(gi, c)] = pos_oh
            pos_oh += noh[(gi, c)]

    # per-edge slot assignment
    key = (core * NB + bucket) * NCLS + cls
    if cfg.get("SRC_SORT"):
        perm = np.lexsort((src, key))
    else:
        perm = np.argsort(key, kind="stable")
    rid = key[perm]
    starts = np.r_[0, np.flatnonzero(np.diff(rid)) + 1]
    counts = np.diff(np.r_[starts, E])
    rank = np.arange(E, dtype=np.int64) - np.repeat(starts, counts)
    slot = sub_start[bucket[perm], cls[perm]] + rank
    core_p = core[perm]

    src_arr = np.zeros((CORES, T * P), dtype=np.int16)   # pads gather row 0
    dst_arr = np.full((CORES, T * P), -1.0, dtype=np.float32)
    src_arr[core_p, slot] = lidx[perm]
    dst_arr[core_p, slot] = din[perm]

    # bucket owner per slot (for ohsrc masking)
    owner = np.full(T * P, -1, dtype=np.int64)
    for b in range(NB):
        for c in range(NCLS):
            if S[b, c]:
                owner[sub_start[b, c]:sub_start[b, c] + S[b, c]] = b

    f16 = mybir.dt.np(mybir.dt.float16)
    idx_np = np.empty((CORES, P, T * 8), dtype=np.int16)
    for c in range(CORES):
        w = src_arr[c].reshape(T * 8, 16).T       # [16, 8T]
        idx_np[c] = np.tile(w, (8, 1))

    # ohsrc[core, p, ohcol]: bucket-local dst of slot (chunk(ohcol), p) if the
    # slot belongs to the column's bucket else -1
    dst3 = dst_arr.reshape(CORES, T, P)           # [core, chunk, p]
    own2 = owner.reshape(T, P)                    # [chunk, p]
    ohsrc = np.full((CORES, P, TOH), -1.0, dtype=np.float32)
    for gi in range(NG):
        for c in range(NCLS):
            t0, _ = units[(gi, c)]
            base = ohoff[(gi, c)]
            for j, (b, tl) in enumerate(ohmap[(gi, c)]):
                tg = t0 + tl
                sel = own2[tg] == b               # [p]
                col = np.where(sel, dst3[:, tg, :], -1.0)
                ohsrc[:, :, base + j] = col
    ohsrc_np = ohsrc.astype(f16)

    meta = dict(NPC=NPC, NB=NB, NG=NG, T=T, sub_start=sub_start, S=S,
                units=units, sched=sched, nckmax=nckmax, nohmax=nohmax,
                noh=noh, ohoff=ohoff, TOH=TOH)
    return S, meta, idx_np, ohsrc_np


def build_nc(S, meta, cfg):
    N, D, CORES, G = cfg["N"], cfg["D"], cfg["CORES"], cfg["GROUP"]
    NB, NG, T = meta["NB"], meta["NG"], meta["T"]
    units, sched = meta["units"], meta["sched"]
    noh, ohoff = meta["noh"], meta["ohoff"]
    NCKMAX, NOHMAX, TOH = meta["nckmax"], meta["nohmax"], meta["TOH"]
    f32 = mybir.dt.float32
    f16 = mybir.dt.float16
    NSWQ = cfg.get("NSWQ", 4)

    # Greedy least-loaded queue assignment: class sub-lists are unbalanced
    # (block0 classes carry ~33% each, block1 ~17%), so round-robin leaves
    # two queues with 1.3x the work.  Balance by accumulated descriptor count.
    _qload = [0] * NSWQ

    def _pick_queue(ndesc):
        q = min(range(NSWQ), key=lambda i: _qload[i])
        _qload[q] += ndesc
        return q

    nc = bacc.Bacc(
        None,
        target_bir_lowering=False,
        dynamic_dma_scratch_size=cfg.get("SCRATCH", 65536),
        num_swdge_queues=NSWQ,
    )
    xb = nc.dram_tensor("xb", [NPAIR, 2 * D], f16, kind="ExternalInput")
    idx_in = nc.dram_tensor("idx", [P, T * 8], mybir.dt.int16, kind="ExternalInput")
    ohsrc_in = nc.dram_tensor("ohsrc", [P, TOH], f16, kind="ExternalInput")
    iota_in = nc.dram_tensor("iota", [P, P, NOHMAX], f16, kind="ExternalInput")
    out = nc.dram_tensor("out", [P, NB * D], f32, kind="ExternalOutput")

    with tile.TileContext(nc) as tc:
        with (
            tc.tile_pool(name="persist", bufs=1) as persist,
            tc.tile_pool(name="stag", bufs=cfg.get("STAG_BUFS", 8)) as stagp,
            tc.tile_pool(name="oh", bufs=cfg.get("OH_BUFS", 4)) as ohp,
            tc.tile_pool(name="psum", bufs=8, space="PSUM") as psump,
        ):
            idx_t = persist.tile([P, T * 8], mybir.dt.int16)
            ohsrc_t = persist.tile([P, TOH], f16)
            iota_t = persist.tile([P, P, NOHMAX], f16)
            outst = persist.tile([P, NB * D], f32)
            nc.sync.dma_start(idx_t[:], idx_in[:])
            nc.sync.dma_start(ohsrc_t[:], ohsrc_in[:])
            nc.sync.dma_start(iota_t[:, :, :], iota_in[:, :, :])

            import contextlib
            reps = cfg.get("REPS", 0)
            loop_cm = tc.For_i(0, reps, 1) if reps else contextlib.nullcontext()
            with loop_cm:
                for gi in range(NG):
                    bks = list(range(gi * G, min((gi + 1) * G, NB)))
                    GA = len(bks)
                    pt = psump.tile([P, G * D], f32, tag="ps", name=f"ps_{gi}")
                    stags, ohs = {}, {}
                    for c in range(4):
                        t0, nck = units[(gi, c)]
                        if nck == 0:
                            continue
                        stag = stagp.tile([P, nck, D], f16, tag="st")
                        stags[c] = stag
                        blk, par = c >> 1, c & 1
                        r0 = blk * PAIR_BLOCK
                        r1 = NPAIR if blk else PAIR_BLOCK
                        if cfg.get("SKIP_GATHER"):
                            nc.gpsimd.memset(stag[:], 0.0)
                        else:
                            gmax = cfg.get("GMAX", 0) or nck
                            for o in range(0, nck, gmax):
                                w = min(gmax, nck - o)
                                q = _pick_queue(w)
                                _dma_gather_small(
                                    nc.gpsimd,
                                    stag[:, o:o + w, :],
                                    xb[r0:r1, par * D:(par + 1) * D],
                                    idx_t[:, (t0 + o) * 8:(t0 + o + w) * 8],
                                    w * P,
                                    w * P,
                                    D,
                                    2 * D,
                                    queue_num=q,
                                    single_packet=False,
                                )
                        # column-major one-hot [part, dstcol, ohcol]: every
                        # operand has a packed 2-byte last dim -> DVE 2x mode
                        nohc = noh[(gi, c)]
                        oh = ohp.tile([P, P, nohc], f16, tag="oh")
                        ohs[c] = oh
                        o0 = ohoff[(gi, c)]
                        if cfg.get("SKIP_OH"):
                            nc.vector.memset(oh[:], 0.0)
                        else:
                            nc.vector.tensor_tensor(
                                out=oh[:],
                                in0=ohsrc_t[:, None, o0:o0 + nohc].to_broadcast(
                                    [P, P, nohc]),
                                in1=iota_t[:, :, :nohc],
                                op=mybir.AluOpType.is_equal,
                            )
                    if cfg.get("SKIP_MM"):
                        if gi == 0:
                            nc.vector.memset(outst[:], 0.0)
                        nc.sync.dma_start(out[:, bks[0] * D:(bks[-1] + 1) * D],
                                          outst[:, bks[0] * D:(bks[-1] + 1) * D])
                        continue
                    for (b, ent) in sched[gi]:
                        h = b - bks[0]
                        for z, (c, tl, ohcol) in enumerate(ent):
                            nc.tensor.matmul(
                                out=pt[:, h * D:(h + 1) * D],
                                lhsT=ohs[c][:, :, ohcol],
                                rhs=stags[c][:, tl, :],
                                start=(z == 0),
                                stop=(z == len(ent) - 1),
                            )
                    c0 = bks[0] * D
                    c1 = (bks[-1] + 1) * D
                    nc.scalar.copy(out=outst[:, c0:c1], in_=pt[:, :GA * D])
                    nc.sync.dma_start(out[:, c0:c1], outst[:, c0:c1])
    nc.finalize()
    return nc


_CACHE = {}


def _get_nc(S, meta, cfg):
    key = (meta["sub_start"].tobytes(), meta["T"], cfg["N"], cfg["D"],
           cfg["CORES"], cfg["GROUP"])
    if key not in _CACHE:
        _CACHE[key] = build_nc(S, meta, cfg)
    return _CACHE[key]


def make_in_maps(x, idx_np, ohsrc_np, cfg, meta):
    CORES = cfg["CORES"]
    f16 = mybir.dt.np(mybir.dt.float16)
    xb = np.ascontiguousarray(
        np.asarray(x, dtype=np.float32).astype(f16).reshape(NPAIR, 2 * cfg["D"])
    )
    iota = np.ascontiguousarray(
        np.broadcast_to(
            np.arange(P, dtype=np.float32)[None, :, None],
            (P, P, meta["nohmax"]),
        ).astype(f16)
    )
    return [
        {"xb": xb, "idx": idx_np[c], "ohsrc": ohsrc_np[c], "iota": iota}
        for c in range(CORES)
    ]


def assemble(shards, meta, cfg):
    N, D, CORES = cfg["N"], cfg["D"], cfg["CORES"]
    NPC, NB = meta["NPC"], meta["NB"]
    full = np.empty((N, D), dtype=np.float32)
    for c in range(CORES):
        arr = shards[c].reshape(P, NB, D).transpose(1, 0, 2).reshape(NB * P, D)
        full[c * NPC:(c + 1) * NPC] = arr[:NPC]
    return full


def kernel(x, edge_index):
    cfg = full_cfg()
    S, meta, idx_np, ohsrc_np = make_layout(edge_index, cfg)
    nc = _get_nc(S, meta, cfg)
    in_maps = make_in_maps(x, idx_np, ohsrc_np, cfg, meta)
    res = run_bass_kernel_spmd(nc, in_maps, core_ids=list(range(cfg["CORES"])))
    shards = [res.results[c]["out"] for c in range(cfg["CORES"])]
    return assemble(shards, meta, cfg)


# revision 9
# speedup vs baseline: 1.1778x; 1.0160x over previous
"""GNN message-passing (gather + segment-sum) Trainium2 kernel.

Strategy (dst-owner sharding, no collectives), v3:
  - Core c owns output nodes [c*NPC, (c+1)*NPC).
  - x is converted to fp16 on HOST and packed as xb[50000, 128]: row k holds
    nodes 2k (cols 0:64) and 2k+1 (cols 64:128).  Device gathers 128B rows at
    256B HBM stride.  A gather call's HBM base picks (block, parity): block =
    pair-index>>15 (int16 index limit), parity = src&1 selects col 0:64/64:128.
  - Gather descriptor cost is ~8.8ns/idx per SWDGE queue (4 queues max) and
    flat in element size, so padded slots are the only lever: sub-lists are
    UNQUANTIZED (size = max over cores of the real count), packed back-to-back
    within each (group, class) unit; only unit tails pad to x128 chunks.
  - Chunks therefore span bucket boundaries.  Instead of tile_position
    (sub-chunk matmuls crash in accumulation chains: FWL row-offset conflict),
    every matmul covers the full 128 slots of a chunk and bucket selection is
    folded into the one-hot: the host prebuilds ohsrc[p, ohcol] = bucket-local
    dst of slot p if the slot belongs to the column's bucket else -1.  A
    boundary chunk gets one ohcol per overlapping bucket.
  - Device per core, per (group, class): one dma_gather -> stag [128, nck, 64]
    fp16; one DVE is_equal builds ALL the unit's one-hot columns COLUMN-MAJOR
    ([part, dstcol, ohcol], packed 2B last dim -> DVE 2x mode) from ohsrc vs
    iota.  PE matmuls BUCKET-major: psum[128 dst, 64 feat] += oh^T @ stag.
    PSUM accumulation groups must be sequential within a bank (HW-verified).
    ACT copies psum[128, G*64] -> SBUF; per-group DMA to HBM out.
  - Output HBM layout [128, NB*64]: (p, b*64+f) = node b*128+p, feature f.
    Host reshapes/transposes the 8 shards -> [100000, 64] f32.
"""

import sys

for _p in ("/opt/trn_rl_repo", "/root/.axon_site/_ro/trn_rl_repo"):
    if _p not in sys.path:
        sys.path.append(_p)

import numpy as np

from concourse import bass, mybir, tile, bacc
from concourse._compat import exact_div, round_up_to_multiple
from concourse.bass_primitives import MemorySpace
from concourse.bass_utils import run_bass_kernel_spmd
import concourse.ap_utils as ap_utils

P = 128
NPAIR = 50000          # pair rows in xb
PAIR_BLOCK = 32768     # pair-index block boundary (int16 range)


def full_cfg():
    return dict(N=100000, D=64, E=1200000, CORES=8, GROUP=8, GMAX=10,
                NSWQ=4, STAG_BUFS=20, OH_BUFS=4, SCRATCH=65536, SRC_SORT=1)


def _dma_gather_small(gp, out_ap, in_ap, idxs_ap, num_idxs, num_idxs_reg,
                      elem_size, elem_step, queue_num=0, single_packet=True):
    """nc.gpsimd.dma_gather minus the elem_size_bytes%256 assert.

    The %256 restriction only applies to the transpose path; the
    non-transpose Q7 desc-gen emits one descriptor of elem_size_bytes per
    index for any size.  The HBM stride (elem_step) must still be x256B.
    """
    gp._assert_queue_num(queue_num)
    assert idxs_ap.dtype == mybir.dt.int16
    assert in_ap.dtype == out_ap.dtype
    elem_size_bytes = elem_size * mybir.dt.size(in_ap.dtype)
    assert elem_size_bytes > 0 and elem_size_bytes % 4 == 0
    assert in_ap.space == MemorySpace.DRAM
    assert idxs_ap.space == MemorySpace.SBUF
    assert out_ap.space == MemorySpace.SBUF
    assert ap_utils.ap_is_contiguous(out_ap.ap[1:])
    assert ap_utils.ap_is_contiguous(idxs_ap.ap[1:])
    assert out_ap.ap[0][1] * out_ap.ap[1][1] == round_up_to_multiple(num_idxs, 128)
    assert in_ap.ap[-1][1] == out_ap.ap[-1][1] == elem_size
    assert in_ap.ap[0][0] == elem_step
    stride_bytes = elem_step * mybir.dt.size(in_ap.dtype)
    stride_bytes_256 = exact_div(stride_bytes, 256)
    assert stride_bytes_256 < 256

    _in_ap = gp.lower_ap_dma(in_ap, for_custom_bir_dma=True)
    _idxs_ap = gp.lower_ap(idxs_ap)
    _out_ap = gp.lower_ap(out_ap)
    return gp.add_instruction(
        mybir.InstDMAGatherAnt(
            name=gp.bass.get_next_instruction_name(),
            ins=[
                *_in_ap,
                _idxs_ap,
                gp.lower_val_access(gp.to_reg(num_idxs_reg)),
            ],
            outs=[_out_ap],
            transpose=False,
            num_idxs=num_idxs,
            elem_size=elem_size,
            stride_bytes_256=stride_bytes_256,
            gen_mode=0,
            single_packet=single_packet,
            queue_num=queue_num,
            sbuf_tokens_per_rank=0,
            sbuf_free_dim_per_rank=0,
            sbuf_free_dim_pad_per_rank=0,
            sbuf_byte_offset=0,
        )
    )


def make_layout(edge_index, cfg):
    """Counting-sort edges into the packed SPMD (group, class, bucket) layout."""
    N, CORES, G = cfg["N"], cfg["CORES"], cfg["GROUP"]
    NPC = N // CORES                      # 12500
    NB = -(-NPC // P)                     # 98 buckets per core
    NCLS = 4
    NG = -(-NB // G)

    src = np.asarray(edge_index[0], dtype=np.int64)
    dst = np.asarray(edge_index[1], dtype=np.int64)
    E = src.shape[0]

    core = dst // NPC
    dstl = dst - core * NPC
    bucket = dstl >> 7
    din = (dstl & 127).astype(np.float32)
    pr = (src & 1)
    half = src >> 1
    blk = (half >= PAIR_BLOCK).astype(np.int64)
    lidx = (half - blk * PAIR_BLOCK).astype(np.int16)
    cls = blk * 2 + pr

    # per-(core, bucket, class) counts -> shared sub-list sizes (no quant)
    cid = (core * NB + bucket) * NCLS + cls
    n = np.bincount(cid, minlength=CORES * NB * NCLS).reshape(CORES, NB, NCLS)
    S = n.max(axis=0).astype(np.int64)            # [NB, NCLS]
    S[:, 0] = np.maximum(S[:, 0], 1)              # every bucket gets >=1 slot

    # layout order: [group][class][bucket in group]; units padded to x128
    sub_start = np.zeros((NB, NCLS), dtype=np.int64)
    units = {}                                    # (gi, c) -> (chunk0, nchunks)
    pos = 0
    for gi in range(NG):
        bks = list(range(gi * G, min((gi + 1) * G, NB)))
        for c in range(NCLS):
            u0 = pos
            for b in bks:
                sub_start[b, c] = pos
                pos += int(S[b, c])
            pos = -(-pos // P) * P
            units[(gi, c)] = (u0 // P, (pos - u0) // P)
    T = pos // P

    # one-hot columns: per (gi, c), per bucket h, one column per chunk its
    # sub-list touches.  sched[gi] = [(b, [(c, tl, ohcol), ...]), ...] with
    # tl local to unit (gi, c) and ohcol local to the unit's oh tile.
    # ohspec[(gi, c)] = [(b, tl, ohcol), ...] duplicates that per-unit view.
    sched = {}
    noh = {}                                      # (gi, c) -> n oh columns
    ohmap = {}                                    # (gi, c) -> list[(b, tl)]
    for gi in range(NG):
        bks = list(range(gi * G, min((gi + 1) * G, NB)))
        glist = []
        cnt = {c: 0 for c in range(NCLS)}
        omap = {c: [] for c in range(NCLS)}
        for b in bks:
            ent = []
            for c in range(NCLS):
                s0, ln = int(sub_start[b, c]), int(S[b, c])
                if ln == 0:
                    continue
                t0, _ = units[(gi, c)]
                tlo = s0 // P - t0
                thi = (s0 + ln - 1) // P - t0
                for tl in range(tlo, thi + 1):
                    ent.append((c, tl, cnt[c]))
                    omap[c].append((b, tl))
                    cnt[c] += 1
            glist.append((b, ent))
        sched[gi] = glist
        for c in range(NCLS):
            noh[(gi, c)] = cnt[c]
            ohmap[(gi, c)] = omap[c]
    nckmax = max(nck for (_, nck) in units.values())
    nohmax = max(noh.values())
    TOH = sum(noh.values())
    ohoff = {}                                    # (gi, c) -> global oh col
    pos_oh = 0
    for gi in range(NG):
        for c in range(NCLS):
            ohoff[(gi, c)] = pos_oh
            pos_oh += noh[(gi, c)]

    # per-edge slot assignment
    key = (core * NB + bucket) * NCLS + cls
    if cfg.get("SRC_SORT"):
        perm = np.lexsort((src, key))
    else:
        perm = np.argsort(key, kind="stable")
    rid = key[perm]
    starts = np.r_[0, np.flatnonzero(np.diff(rid)) + 1]
    counts = np.diff(np.r_[starts, E])
    rank = np.arange(E, dtype=np.int64) - np.repeat(starts, counts)
    slot = sub_start[bucket[perm], cls[perm]] + rank
    core_p = core[perm]

    src_arr = np.zeros((CORES, T * P), dtype=np.int16)   # pads gather row 0
    dst_arr = np.full((CORES, T * P), -1.0, dtype=np.float32)
    src_arr[core_p, slot] = lidx[perm]
    dst_arr[core_p, slot] = din[perm]

    # bucket owner per slot (for ohsrc masking)
    owner = np.full(T * P, -1, dtype=np.int64)
    for b in range(NB):
        for c in range(NCLS):
            if S[b, c]:
                owner[sub_start[b, c]:sub_start[b, c] + S[b, c]] = b

    f16 = mybir.dt.np(mybir.dt.float16)
    idx_np = np.empty((CORES, P, T * 8), dtype=np.int16)
    for c in range(CORES):
        w = src_arr[c].reshape(T * 8, 16).T       # [16, 8T]
        idx_np[c] = np.tile(w, (8, 1))

    # ohsrc[core, p, ohcol]: bucket-local dst of slot (chunk(ohcol), p) if the
    # slot belongs to the column's bucket else -1
    dst3 = dst_arr.reshape(CORES, T, P)           # [core, chunk, p]
    own2 = owner.reshape(T, P)                    # [chunk, p]
    ohsrc = np.full((CORES, P, TOH), -1.0, dtype=np.float32)
    for gi in range(NG):
        for c in range(NCLS):
            t0, _ = units[(gi, c)]
            base = ohoff[(gi, c)]
            for j, (b, tl) in enumerate(ohmap[(gi, c)]):
                tg = t0 + tl
                sel = own2[tg] == b               # [p]
                col = np.where(sel, dst3[:, tg, :], -1.0)
                ohsrc[:, :, base + j] = col
    ohsrc_np = ohsrc.astype(f16)

    meta = dict(NPC=NPC, NB=NB, NG=NG, T=T, sub_start=sub_start, S=S,
                units=units, sched=sched, nckmax=nckmax, nohmax=nohmax,
                noh=noh, ohoff=ohoff, TOH=TOH)
    return S, meta, idx_np, ohsrc_np


def build_nc(S, meta, cfg):
    N, D, CORES, G = cfg["N"], cfg["D"], cfg["CORES"], cfg["GROUP"]
    NB, NG, T = meta["NB"], meta["NG"], meta["T"]
    units, sched = meta["units"], meta["sched"]
    noh, ohoff = meta["noh"], meta["ohoff"]
    NCKMAX, NOHMAX, TOH = meta["nckmax"], meta["nohmax"], meta["TOH"]
    f32 = mybir.dt.float32
    f16 = mybir.dt.float16
    NSWQ = cfg.get("NSWQ", 4)

    # Greedy least-loaded queue assignment: class sub-lists are unbalanced
    # (block0 classes carry ~33% each, block1 ~17%), so round-robin leaves
    # two queues with 1.3x the work.  Balance by accumulated descriptor count.
    _qload = [0] * NSWQ

    def _pick_queue(ndesc):
        q = min(range(NSWQ), key=lambda i: _qload[i])
        _qload[q] += ndesc
        return q

    nc = bacc.Bacc(
        None,
        target_bir_lowering=False,
        dynamic_dma_scratch_size=cfg.get("SCRATCH", 65536),
        num_swdge_queues=NSWQ,
    )
    xb = nc.dram_tensor("xb", [NPAIR, 2 * D], f16, kind="ExternalInput")
    idx_in = nc.dram_tensor("idx", [P, T * 8], mybir.dt.int16, kind="ExternalInput")
    ohsrc_in = nc.dram_tensor("ohsrc", [P, TOH], f16, kind="ExternalInput")
    iota_in = nc.dram_tensor("iota", [P, P, NOHMAX], f16, kind="ExternalInput")
    out = nc.dram_tensor("out", [P, NB * D], f32, kind="ExternalOutput")

    with tile.TileContext(nc) as tc:
        with (
            tc.tile_pool(name="persist", bufs=1) as persist,
            tc.tile_pool(name="stag", bufs=cfg.get("STAG_BUFS", 8)) as stagp,
            tc.tile_pool(name="oh", bufs=cfg.get("OH_BUFS", 4)) as ohp,
            tc.tile_pool(name="psum", bufs=8, space="PSUM") as psump,
        ):
            idx_t = persist.tile([P, T * 8], mybir.dt.int16)
            ohsrc_t = persist.tile([P, TOH], f16)
            iota_t = persist.tile([P, P, NOHMAX], f16)
            outst = persist.tile([P, NB * D], f32)
            nc.sync.dma_start(idx_t[:], idx_in[:])
            nc.sync.dma_start(ohsrc_t[:], ohsrc_in[:])
            nc.sync.dma_start(iota_t[:, :, :], iota_in[:, :, :])

            import contextlib
            reps = cfg.get("REPS", 0)
            loop_cm = tc.For_i(0, reps, 1) if reps else contextlib.nullcontext()
            with loop_cm:
                for gi in range(NG):
                    bks = list(range(gi * G, min((gi + 1) * G, NB)))
                    GA = len(bks)
                    pt = psump.tile([P, G * D], f32, tag="ps", name=f"ps_{gi}")
                    stags, ohs = {}, {}
                    for c in range(4):
                        t0, nck = units[(gi, c)]
                        if nck == 0:
                            continue
                        # one tile PER GATHER SUB-CALL: slices of a shared
                        # tile would serialize on tile-level dependency
                        # tracking; separate tiles let calls pipeline.
                        gmax = cfg.get("GMAX", 0) or nck
                        tl_tiles = []
                        blk, par = c >> 1, c & 1
                        r0 = blk * PAIR_BLOCK
                        r1 = NPAIR if blk else PAIR_BLOCK
                        for o in range(0, nck, gmax):
                            w = min(gmax, nck - o)
                            stag = stagp.tile([P, w, D], f16, tag="st")
                            tl_tiles.append((o, stag))
                            if cfg.get("SKIP_GATHER"):
                                nc.gpsimd.memset(stag[:], 0.0)
                                continue
                            q = _pick_queue(w)
                            _dma_gather_small(
                                nc.gpsimd,
                                stag[:],
                                xb[r0:r1, par * D:(par + 1) * D],
                                idx_t[:, (t0 + o) * 8:(t0 + o + w) * 8],
                                w * P,
                                w * P,
                                D,
                                2 * D,
                                queue_num=q,
                                single_packet=False,
                            )
                        stags[c] = (gmax, tl_tiles)
                        # column-major one-hot [part, dstcol, ohcol]: every
                        # operand has a packed 2-byte last dim -> DVE 2x mode
                        nohc = noh[(gi, c)]
                        oh = ohp.tile([P, P, nohc], f16, tag="oh")
                        ohs[c] = oh
                        o0 = ohoff[(gi, c)]
                        if cfg.get("SKIP_OH"):
                            nc.vector.memset(oh[:], 0.0)
                        else:
                            nc.vector.tensor_tensor(
                                out=oh[:],
                                in0=ohsrc_t[:, None, o0:o0 + nohc].to_broadcast(
                                    [P, P, nohc]),
                                in1=iota_t[:, :, :nohc],
                                op=mybir.AluOpType.is_equal,
                            )
                    if cfg.get("SKIP_MM"):
                        if gi == 0:
                            nc.vector.memset(outst[:], 0.0)
                        nc.sync.dma_start(out[:, bks[0] * D:(bks[-1] + 1) * D],
                                          outst[:, bks[0] * D:(bks[-1] + 1) * D])
                        continue
                    for (b, ent) in sched[gi]:
                        h = b - bks[0]
                        for z, (c, tl, ohcol) in enumerate(ent):
                            gmax, tl_tiles = stags[c]
                            stag = tl_tiles[tl // gmax][1]
                            nc.tensor.matmul(
                                out=pt[:, h * D:(h + 1) * D],
                                lhsT=ohs[c][:, :, ohcol],
                                rhs=stag[:, tl % gmax, :],
                                start=(z == 0),
                                stop=(z == len(ent) - 1),
                            )
                    c0 = bks[0] * D
                    c1 = (bks[-1] + 1) * D
                    nc.scalar.copy(out=outst[:, c0:c1], in_=pt[:, :GA * D])
                    nc.sync.dma_start(out[:, c0:c1], outst[:, c0:c1])
    nc.finalize()
    return nc


_CACHE = {}


def _get_nc(S, meta, cfg):
    key = (meta["sub_start"].tobytes(), meta["T"], cfg["N"], cfg["D"],
           cfg["CORES"], cfg["GROUP"])
    if key not in _CACHE:
        _CACHE[key] = build_nc(S, meta, cfg)
    return _CACHE[key]


def make_in_maps(x, idx_np, ohsrc_np, cfg, meta):
    CORES = cfg["CORES"]
    f16 = mybir.dt.np(mybir.dt.float16)
    xb = np.ascontiguousarray(
        np.asarray(x, dtype=np.float32).astype(f16).reshape(NPAIR, 2 * cfg["D"])
    )
    iota = np.ascontiguousarray(
        np.broadcast_to(
            np.arange(P, dtype=np.float32)[None, :, None],
            (P, P, meta["nohmax"]),
        ).astype(f16)
    )
    return [
        {"xb": xb, "idx": idx_np[c], "ohsrc": ohsrc_np[c], "iota": iota}
        for c in range(CORES)
    ]


def assemble(shards, meta, cfg):
    N, D, CORES = cfg["N"], cfg["D"], cfg["CORES"]
    NPC, NB = meta["NPC"], meta["NB"]
    full = np.empty((N, D), dtype=np.float32)
    for c in range(CORES):
        arr = shards[c].reshape(P, NB, D).transpose(1, 0, 2).reshape(NB * P, D)
        full[c * NPC:(c + 1) * NPC] = arr[:NPC]
    return full


def kernel(x, edge_index):
    cfg = full_cfg()
    S, meta, idx_np, ohsrc_np = make_layout(edge_index, cfg)
    nc = _get_nc(S, meta, cfg)
    in_maps = make_in_maps(x, idx_np, ohsrc_np, cfg, meta)
    res = run_bass_kernel_spmd(nc, in_maps, core_ids=list(range(cfg["CORES"])))
    shards = [res.results[c]["out"] for c in range(cfg["CORES"])]
    return assemble(shards, meta, cfg)
